# revision 1
# baseline (speedup 1.0000x reference)
"""Trainium2 kernel for nn_MHAttention_15358803050646.

The reference module computes
    qkv = qkv_w @ x + qkv_b          (1x1 conv over channels)
    q, k, v = split(qkv)
    att = softmax(q @ k^T / sqrt(d_k))
    out = einsum('bnqk,bnqd->bnqd', att, v)      # <-- sums att over k
    out = out_w @ out + out_b

The einsum 'bnqk,bnqd->bnqd' multiplies v elementwise by the softmax
row-sum, which is identically 1.  The whole attention block is therefore
the identity on v, and the network collapses algebraically to

    out = out_w @ (v_w @ x + v_b) + out_b = W_eff @ x + b_eff

with v_w = qkv_w[1024:1536], v_b = qkv_b[1024:1536].  We fuse the two
channel matrices on the host (512x512x512 fp32, sub-millisecond) and run
a single 512x512 channel projection over all pixels on device.

Sharding: data-parallel over batch — B == 8 images, one per NeuronCore.
Per core: out[o, p] = sum_c W_eff[o, c] * x[c, p] + b_eff[o] with
C = 512 channels and HW = 1024 pixels, i.e. a 512x512x1024 matmul.

Matmul precision ("fp16x2" mode, default): the TRN2 PE runs fp32 matmuls
at 4 cycles/row but fp16 at 1 cycle/row.  Each fp32 operand is split on
the host into an fp16 high part and an fp16 residual (hi = fp16(a),
lo = fp16(a - hi)); the product is computed as three fp16 matmuls
Wh@Xh + Wh@Xl + Wl@Xh accumulated in fp32 PSUM.  fp16 has 11 mantissa
bits, so hi+lo carries ~22 bits and the dropped Wl@Xl term is O(2^-24)
relative: measured end-to-end relative error is ~4e-7, the same as a
plain fp32 matmul, at 3/4 the PE cost and identical DMA bytes.

Device layouts are packed on the host so every DMA is 128 partitions x
contiguous bytes per partition.
"""

import numpy as np

import concourse.mybir as mybir
import concourse.tile as tile
from concourse import bacc
from concourse.bass_utils import run_bass_kernel_spmd

P = 128          # SBUF partitions
C = 512          # model channels
HW = 1024        # pixels per image (32*32)
B = 8            # batch == number of cores
KO = C // P      # contraction chunks (4)
MO = C // P      # output-channel chunks (4)
N_TILE = 512     # pixels per PSUM tile (one fp32 PSUM bank)
N_TILES = HW // N_TILE

_FP32 = mybir.dt.float32
_FP16 = mybir.dt.float16


def _build_fp16x2(nc):
    """3-term fp16 split-matmul kernel body (see module docstring).

    Schedule notes (cost-model driven):
    - All DMA transfers serialize on the shared SDMA engines (~360 GB/s), so
      the first matmul can only start once its operands' transfers finish.
      The n=0 operands are therefore loaded in P-sized k-chunks, interleaved
      hi-before-lo in the order the PE consumes them, letting PE start after
      ~256 KB instead of ~2 MB.
    - Input DMAs are issued from several engines (SP: hi stream, ACT: lo
      stream + bias, Pool/SWDGE: trailing lo tile) so per-DMA issue cost does
      not serialize behind one sequencer.
    - Output DMAs are issued from the Activation engine: each directly
      follows its bias-add activation in program order, needing no semaphore.
    - n=0 uses k-outer matmul order (stream-friendly); the last n-tile uses
      m-outer order so the four output groups finish staggered and the tail
      only waits for one small DMA.
    """
    wh = nc.declare_dram_parameter("wh", [P, KO * C], _FP16, isOutput=False)
    wl = nc.declare_dram_parameter("wl", [P, KO * C], _FP16, isOutput=False)
    bias = nc.declare_dram_parameter("bias", [P, MO], _FP32, isOutput=False)
    # x*[n*P + p, ko*N_TILE + j] = x_core[ko*P + p, n*N_TILE + j]
    xh = nc.declare_dram_parameter("xh", [N_TILES * P, KO * N_TILE], _FP16, isOutput=False)
    xl = nc.declare_dram_parameter("xl", [N_TILES * P, KO * N_TILE], _FP16, isOutput=False)
    # out[(n*MO + m)*P + p, j] = out_core[m*P + p, n*N_TILE + j]
    out = nc.declare_dram_parameter("out", [N_TILES * MO * P, N_TILE], _FP32, isOutput=True)

    wh_r = wh.rearrange("p (ko o) -> p ko o", ko=KO)
    wl_r = wl.rearrange("p (ko o) -> p ko o", ko=KO)

    with tile.TileContext(nc) as tc:
        with (
            tc.tile_pool(name="wpool", bufs=1) as wpool,
            tc.tile_pool(name="xpool", bufs=2) as xpool,
            tc.tile_pool(name="opool", bufs=4) as opool,
            tc.tile_pool(name="psum", bufs=8, space="PSUM") as psum_pool,
        ):
            b_sb = wpool.tile([P, MO], _FP32, tag="bias")
            nc.scalar.dma_start(b_sb[:], bias[:])

            # n=0 operands, k-chunked, in PE consumption order.
            wh_k = [wpool.tile([P, C], _FP16, tag=f"wh{k}", name=f"wh_k{k}") for k in range(KO)]
            wl_k = [wpool.tile([P, C], _FP16, tag=f"wl{k}", name=f"wl_k{k}") for k in range(KO)]
            xh0_k = [xpool.tile([P, N_TILE], _FP16, tag=f"xh0_{k}", name=f"xh0_k{k}") for k in range(KO)]
            xl0_k = [xpool.tile([P, N_TILE], _FP16, tag=f"xl0_{k}", name=f"xl0_k{k}") for k in range(KO)]
            for k in range(KO):
                nc.sync.dma_start(wh_k[k][:], wh_r[:, k])
                nc.sync.dma_start(xh0_k[k][:], xh[0:P, k * N_TILE:(k + 1) * N_TILE])
            for k in range(KO):
                nc.scalar.dma_start(wl_k[k][:], wl_r[:, k])
                nc.scalar.dma_start(xl0_k[k][:], xl[0:P, k * N_TILE:(k + 1) * N_TILE])

            # Remaining n-tiles: whole-tile loads (they arrive long before use).
            x_rest = []
            for n in range(1, N_TILES):
                xh_sb = xpool.tile([P, KO, N_TILE], _FP16, tag="xh")
                nc.sync.dma_start(
                    xh_sb[:], xh[n * P:(n + 1) * P].rearrange("p (ko j) -> p ko j", ko=KO))
                xl_sb = xpool.tile([P, KO, N_TILE], _FP16, tag="xl")
                nc.gpsimd.dma_start(
                    xl_sb[:], xl[n * P:(n + 1) * P].rearrange("p (ko j) -> p ko j", ko=KO))
                x_rest.append((xh_sb, xl_sb))

            def emit_group_tail(n, m, ps):
                o_sb = opool.tile([P, N_TILE], _FP32, tag="o")
                row = (n * MO + m) * P
                # out = psum + bias[o], PSUM -> SBUF on the scalar engine
                nc.scalar.activation(
                    o_sb[:], ps[:], mybir.ActivationFunctionType.Identity,
                    bias=b_sb[:, m:m + 1])
                nc.scalar.dma_start(out[row:row + P], o_sb[:])

            # n=0: k-outer, hi terms first, 4 psum groups in flight.
            ps0 = [psum_pool.tile([P, N_TILE], _FP32, tag="ps", name=f"ps0_{m}") for m in range(MO)]
            for k in range(KO):
                for m in range(MO):
                    nc.tensor.matmul(ps0[m][:], lhsT=wh_k[k][:, m * P:(m + 1) * P],
                                     rhs=xh0_k[k][:], start=(k == 0), stop=False)
            for k in range(KO):
                for m in range(MO):
                    nc.tensor.matmul(ps0[m][:], lhsT=wl_k[k][:, m * P:(m + 1) * P],
                                     rhs=xh0_k[k][:], start=False, stop=False)
            for k in range(KO):
                for m in range(MO):
                    nc.tensor.matmul(ps0[m][:], lhsT=wh_k[k][:, m * P:(m + 1) * P],
                                     rhs=xl0_k[k][:], start=False, stop=(k == KO - 1))
            for m in range(MO):
                emit_group_tail(0, m, ps0[m])

            # n>=1: m-outer so groups retire staggered.  The very last
            # m-group runs as two half-width (N/2) psum groups: the final
            # ACT -> out-DMA chain is then half-length and starts 12
            # half-matmuls earlier, trimming the kernel tail.
            for n in range(1, N_TILES):
                xh_sb, xl_sb = x_rest[n - 1]
                for m in range(MO):
                    om = slice(m * P, (m + 1) * P)
                    last_group = n == N_TILES - 1 and m == MO - 1
                    halves = (
                        [(slice(0, N_TILE // 2), 0), (slice(N_TILE // 2, N_TILE), 1)]
                        if last_group else [(slice(0, N_TILE), None)]
                    )
                    for js, half in halves:
                        ps = psum_pool.tile([P, js.stop - js.start], _FP32, tag="ps",
                                            name=f"ps_{n}_{m}_{half}")
                        for k in range(KO):
                            nc.tensor.matmul(ps[:], lhsT=wh_k[k][:, om],
                                             rhs=xh_sb[:, k, js],
                                             start=(k == 0), stop=False)
                        for k in range(KO):
                            nc.tensor.matmul(ps[:], lhsT=wl_k[k][:, om],
                                             rhs=xh_sb[:, k, js],
                                             start=False, stop=False)
                        for k in range(KO):
                            nc.tensor.matmul(ps[:], lhsT=wh_k[k][:, om],
                                             rhs=xl_sb[:, k, js],
                                             start=False, stop=(k == KO - 1))
                        o_sb = opool.tile([P, js.stop - js.start], _FP32, tag="o",
                                          name=f"o_{n}_{m}_{half}")
                        nc.scalar.activation(
                            o_sb[:], ps[:], mybir.ActivationFunctionType.Identity,
                            bias=b_sb[:, m:m + 1])
                        row = (n * MO + m) * P
                        if half == 0:
                            # keep ACT's sequencer free for the final
                            # activation: the first half's store goes via SP
                            nc.sync.dma_start(out[row:row + P, js], o_sb[:])
                        else:
                            nc.scalar.dma_start(out[row:row + P, js], o_sb[:])


def _build_fp32(nc, mm_dtype):
    """Single-dtype kernel body (fp32 or f32r matmuls)."""
    w = nc.declare_dram_parameter("w", [P, KO * C], mm_dtype, isOutput=False)
    bias = nc.declare_dram_parameter("bias", [P, MO], _FP32, isOutput=False)
    x = nc.declare_dram_parameter("x", [N_TILES * P, KO * N_TILE], mm_dtype, isOutput=False)
    out = nc.declare_dram_parameter("out", [N_TILES * MO * P, N_TILE], _FP32, isOutput=True)

    with tile.TileContext(nc) as tc:
        with (
            tc.tile_pool(name="wpool", bufs=1) as wpool,
            tc.tile_pool(name="xpool", bufs=N_TILES) as xpool,
            tc.tile_pool(name="opool", bufs=4) as opool,
            tc.tile_pool(name="psum", bufs=8, space="PSUM") as psum_pool,
        ):
            w_sb = wpool.tile([P, KO, C], mm_dtype, tag="w")
            nc.sync.dma_start(w_sb[:], w.rearrange("p (ko o) -> p ko o", ko=KO))
            x_sbs = []
            for n in range(N_TILES):
                x_sb = xpool.tile([P, KO, N_TILE], mm_dtype, tag="x")
                nc.sync.dma_start(
                    x_sb[:], x[n * P:(n + 1) * P].rearrange("p (ko j) -> p ko j", ko=KO))
                x_sbs.append(x_sb)
                if n == 0:
                    b_sb = wpool.tile([P, MO], _FP32, tag="bias")
                    nc.sync.dma_start(b_sb[:], bias[:])

            for n in range(N_TILES):
                x_sb = x_sbs[n]
                for m in range(MO):
                    ps = psum_pool.tile([P, N_TILE], _FP32, tag="ps")
                    for k in range(KO):
                        nc.tensor.matmul(
                            ps[:], lhsT=w_sb[:, k, m * P:(m + 1) * P], rhs=x_sb[:, k, :],
                            start=(k == 0), stop=(k == KO - 1))
                    o_sb = opool.tile([P, N_TILE], _FP32, tag="o")
                    nc.scalar.activation(
                        o_sb[:], ps[:], mybir.ActivationFunctionType.Identity,
                        bias=b_sb[:, m:m + 1])
                    nc.sync.dma_start(out[(n * MO + m) * P:(n * MO + m + 1) * P], o_sb[:])


def _build_bass(mode="fp16x2"):
    # Bacc (not plain Bass): its finalize() runs the legalization passes that
    # split multi-semaphore waits (TRN2 allows one sync wait per instruction).
    nc = bacc.Bacc()
    if mode == "fp16x2":
        _build_fp16x2(nc)
    elif mode == "fp32":
        _build_fp32(nc, _FP32)
    elif mode == "f32r":
        _build_fp32(nc, mybir.dt.float32r)
    else:
        raise ValueError(mode)
    # Runs Bacc.compile(): moves matmul waits to ldweights, splits multi-sem
    # waits into event semaphores, allocates registers.
    nc.finalize()
    return nc


def _pack_w(w2d):
    # [C, C] (transposed W_eff: w2d[c, o]) -> [P, KO*C] with [p, ko, o] layout
    return np.ascontiguousarray(
        w2d.reshape(KO, P, C).transpose(1, 0, 2)).reshape(P, KO * C)


def _pack_x(xm):
    # [B, C, HW] -> [B, N_TILES*P, KO*N_TILE] with [n, p, ko, j] layout
    t = xm.reshape(B, KO, P, N_TILES, N_TILE).transpose(0, 3, 2, 1, 4)
    return np.ascontiguousarray(t).reshape(B, N_TILES * P, KO * N_TILE)


_NC_CACHE = {}


def _get_nc(mode):
    if mode not in _NC_CACHE:
        _NC_CACHE[mode] = _build_bass(mode)
    return _NC_CACHE[mode]


MODE = "fp16x2"


def kernel(x, qkv_w, qkv_b, out_w, out_b):
    x = np.asarray(x, dtype=np.float32)
    qkv_w = np.asarray(qkv_w, dtype=np.float32)
    qkv_b = np.asarray(qkv_b, dtype=np.float32)
    out_w = np.asarray(out_w, dtype=np.float32)
    out_b = np.asarray(out_b, dtype=np.float32)

    Bx, Cx, Hx, Wx = x.shape
    assert (Bx, Cx, Hx * Wx) == (B, C, HW), (x.shape,)

    # Host-side algebraic fusion (see module docstring).
    v_w = qkv_w[2 * C:3 * C]
    v_b = qkv_b[2 * C:3 * C]
    w_eff = out_w @ v_w                    # [C, C]
    b_eff = out_w @ v_b + out_b            # [C]

    bias_host = np.ascontiguousarray(b_eff.reshape(MO, P).T)
    xm = x.reshape(B, C, HW)
    wt = np.ascontiguousarray(w_eff.T)     # wt[c, o]

    nc = _get_nc(MODE)
    if MODE == "fp16x2":
        wt_h = wt.astype(np.float16)
        wt_l = (wt - wt_h.astype(np.float32)).astype(np.float16)
        x_h16 = xm.astype(np.float16)
        x_l16 = (xm - x_h16.astype(np.float32)).astype(np.float16)
        wh_host = _pack_w(wt_h)
        wl_host = _pack_w(wt_l)
        xh_host = _pack_x(x_h16)
        xl_host = _pack_x(x_l16)
        in_maps = [
            {"wh": wh_host, "wl": wl_host, "bias": bias_host,
             "xh": xh_host[i], "xl": xl_host[i]}
            for i in range(B)
        ]
    else:
        w_host = _pack_w(wt)
        x_dev = _pack_x(xm)
        in_maps = [{"w": w_host, "bias": bias_host, "x": x_dev[i]} for i in range(B)]

    res = run_bass_kernel_spmd(nc, in_maps, core_ids=list(range(B)))

    # out rows [(n*MO + m)*P + p] hold out_core[m*P + p, n*N_TILE:(n+1)*N_TILE]
    out_dev = np.stack([res.results[i]["out"] for i in range(B)], axis=0)
    out_dev = out_dev.reshape(B, N_TILES, MO, P, N_TILE)
    out_full = out_dev.transpose(0, 2, 3, 1, 4).reshape(B, C, Hx, Wx)
    return np.ascontiguousarray(out_full.astype(np.float32))



# revision 3
# speedup vs baseline: 2.6290x; 2.6290x over previous
"""Trainium2 kernel for nn_MHAttention_15358803050646.

The reference module computes
    qkv = qkv_w @ x + qkv_b          (1x1 conv over channels)
    q, k, v = split(qkv)
    att = softmax(q @ k^T / sqrt(d_k))
    out = einsum('bnqk,bnqd->bnqd', att, v)      # <-- sums att over k
    out = out_w @ out + out_b

The einsum 'bnqk,bnqd->bnqd' multiplies v elementwise by the softmax
row-sum, which is identically 1.  The whole attention block is therefore
the identity on v, and the network collapses algebraically to

    out = out_w @ (v_w @ x + v_b) + out_b = W_eff @ x + b_eff

with v_w = qkv_w[1024:1536], v_b = qkv_b[1024:1536].  We fuse the two
channel matrices on the host (512x512x512 fp32, sub-millisecond) and run
a single 512x512 channel projection over all pixels on device.

Sharding: data-parallel over batch — B == 8 images, one per NeuronCore.
Per core: out[o, p] = sum_c W_eff[o, c] * x[c, p] + b_eff[o] with
C = 512 channels and HW = 1024 pixels, i.e. a 512x512x1024 matmul.

Kernel design (mode "fp16s", cost-model driven):
- Single fp16 matmul pass.  fp16 carries 11 mantissa bits; with fp32 PSUM
  accumulation the end-to-end relative error is ~4e-4, far inside the 2e-2
  gate, at 1 PE cycle/row (fp32 needs 4, a hi/lo fp16 split needs 3).
- fp16 outputs; the host upcasts to fp32.  Halves the store traffic.
- Inputs stream on the SP queue as four fused [w_k | x_k] 256KB chunks so
  one DMA enables a full contraction round, then the two x-halves of the
  second pixel tile.
- 5 warmup matmuls on a memset tile keep the PE busy from ~0.5us so the
  p-state ramp (0.65 -> 1.2 -> 2.4 GHz after 3us continuous busy) is over
  as early as possible; real matmuls start as soon as the first chunk
  lands (~1.0us).
- Bias-add + PSUM->SBUF fp16 downcast alternates between the Activation
  and Vector engines so neither becomes the drain bottleneck.
- All output stores go through SWDGE prepare_only scatter-add descriptors
  (generated early on the idle Pool engine) fired by trigger_dma right
  after each staging drain.  This skips the HWDGE+DGE store pipeline
  (~1.2us) at the tail.  Scatter-add accumulates, so the out buffer is
  zero-filled first by two 512KB DMAs from a memset tile (off the
  critical path).  The last m-group is column-split 320/192 with drains
  balanced across ACT/DVE so the final trigger fires ~320ns after the
  last matmul.
"""

import numpy as np

import concourse.mybir as mybir
import concourse.tile as tile
from concourse import bacc
from concourse.bass_utils import run_bass_kernel_spmd

P = 128          # SBUF partitions
C = 512          # model channels
HW = 1024        # pixels per image (32*32)
B = 8            # batch == number of cores
KO = C // P      # contraction chunks (4)
MO = C // P      # output-channel chunks (4)
N_TILE = 512     # pixels per PSUM tile (one fp32 PSUM bank)
N_TILES = HW // N_TILE

_FP32 = mybir.dt.float32
_FP16 = mybir.dt.float16

WARM_INSTS = 5   # warmup matmuls (128 cols each) before real operands land
WARM_COLS = 128
M3A = 320        # column split of the last m-group (ACT 320 | DVE 192)


def _build_fp16s(nc):
    """Single-pass fp16 kernel body (see module docstring)."""
    # Fused per-k chunks: [w_k (C cols) | x_n0_k (N_TILE cols)] so one DMA
    # enables a full k-round of the first pixel tile.
    wx = nc.declare_dram_parameter("wx", [P, KO * (C + N_TILE)], _FP16, isOutput=False)
    x1 = nc.declare_dram_parameter("x1", [P, KO * N_TILE], _FP16, isOutput=False)
    bias = nc.declare_dram_parameter("bias", [P, MO], _FP32, isOutput=False)
    # scatter index table, replicated over 16-partition groups:
    # sidx[p, s] = 16*s + (p % 16).  Host-provided: the device SWDGE ucode
    # reads the replicated layout (an iota over partitions 0:16 with -1
    # padding crashes it).
    sidx = nc.declare_dram_parameter("sidx", [P, 16], mybir.dt.int16, isOutput=False)
    # out[(n*MO + m)*P + p, j] = out_core[m*P + p, n*N_TILE + j]
    out = nc.declare_dram_parameter("out", [N_TILES * MO * P, N_TILE], _FP16, isOutput=True)

    wx_r = wx.rearrange("p (ko c) -> p ko c", ko=KO)
    x1_r = x1.rearrange("p (ko j) -> p ko j", ko=KO)
    ha, hb = M3A, N_TILE - M3A

    with tile.TileContext(nc) as tc:
        with (
            tc.tile_pool(name="wpool", bufs=1) as wpool,
            tc.tile_pool(name="opool", bufs=1) as opool,
            tc.tile_pool(name="psum", bufs=8, space="PSUM") as psum_pool,
        ):
            # --- warmup: PE busy from ~t=0.5us so the p-state ramp is done
            # by t=3.3us.
            wz = wpool.tile([P, WARM_COLS], _FP16, tag="wz")
            nc.vector.memset(wz[:], 0.03125)
            ps_warm = psum_pool.tile([P, WARM_COLS], _FP32, tag="ps", name="ps_warm")
            for _ in range(WARM_INSTS):
                nc.tensor.matmul(ps_warm[:], lhsT=wz[:, :P], rhs=wz[:],
                                 start=True, stop=True)

            # --- bias on the Pool queue (SWDGE), keeps HWDGE free.
            b_sb = wpool.tile([P, MO], _FP32, tag="bias")
            nc.gpsimd.dma_start(b_sb[:], bias[:])

            # --- input stream on the SP queue (HWDGE).
            wx_sb = [wpool.tile([P, C + N_TILE], _FP16, tag=f"wx{k}", name=f"wx{k}")
                     for k in range(KO)]
            for k in range(KO):
                nc.sync.dma_start(wx_sb[k][:], wx_r[:, k])
            x1_sb = wpool.tile([P, KO, N_TILE], _FP16, tag="x1")
            nc.sync.dma_start(x1_sb[:, 0:2], x1_r[:, 0:2])
            nc.sync.dma_start(x1_sb[:, 2:4], x1_r[:, 2:4])

            def lhsT(k, m):
                return wx_sb[k][:, m * P:(m + 1) * P]

            # --- n0 (pixels 0..511): k-outer (stream-friendly), m-inner.
            ps0 = [psum_pool.tile([P, N_TILE], _FP32, tag="ps", name=f"ps0_{m}")
                   for m in range(MO)]
            for k in range(KO):
                for m in range(MO):
                    nc.tensor.matmul(ps0[m][:], lhsT=lhsT(k, m),
                                     rhs=wx_sb[k][:, C:],
                                     start=(k == 0), stop=(k == KO - 1))

            # n0 drain: ACT m0/m2, DVE m1/m3, into one staging tile.
            o0 = opool.tile([P, MO, N_TILE], _FP16, tag="o0")
            nc.scalar.activation(o0[:, 0], ps0[0][:],
                                 mybir.ActivationFunctionType.Identity,
                                 bias=b_sb[:, 0:1])
            nc.vector.tensor_scalar_add(o0[:, 1], ps0[1][:], b_sb[:, 1:2])
            nc.scalar.activation(o0[:, 2], ps0[2][:],
                                 mybir.ActivationFunctionType.Identity,
                                 bias=b_sb[:, 2:3])
            nc.vector.tensor_scalar_add(o0[:, 3], ps0[3][:], b_sb[:, 3:4])

            # --- n1 staging tiles (scatter sources must be [128, g, elem]).
            o1a = opool.tile([P, 2, N_TILE], _FP16, tag="o1a")
            o1b = opool.tile([P, 1, N_TILE], _FP16, tag="o1b")
            o1ca = opool.tile([P, 1, ha], _FP16, tag="o1ca")
            o1cb = opool.tile([P, 1, hb], _FP16, tag="o1cb")

            # scatter indices (see sidx declaration).
            idx = opool.tile([P, 16], mybir.dt.int16, tag="sidx")
            nc.gpsimd.dma_start(idx[:], sidx[:])

            # scatter-add accumulates, so zero-fill the whole out buffer
            # first: two 512KB DMAs from a memset tile, early on the Pool
            # queue.  The preps' WAW attribution orders the scattered
            # writes after these.
            zt = opool.tile([P, 2, N_TILE], _FP16, tag="zt")
            nc.vector.memset(zt[:], 0)
            for zb in range(0, 2 * MO, 2):
                nc.gpsimd.dma_start(
                    out[zb * P:(zb + 2) * P].rearrange("(mo p) j -> p mo j", p=P),
                    zt[:])

            # n0 preps on queue 0, fired as soon as the n0 drains land.
            # (o1a reuses queue 0 afterwards: its prep is emitted after this
            # trigger, so the pending list is empty again.)
            sems = [nc.alloc_semaphore(f"sc{i}") for i in range(7)]
            nc.gpsimd.dma_scatter_add(
                out[0:2 * P], o0[:, 0:2], idx[:, 0:16], 256, 256, N_TILE,
                prepare_only=True, sem=sems[0], queue_num=0)
            nc.gpsimd.dma_scatter_add(
                out[2 * P:MO * P], o0[:, 2:4], idx[:, 0:16], 256, 256, N_TILE,
                prepare_only=True, sem=sems[1], queue_num=0)
            nc.gpsimd.trigger_dma(count=None, queue_num=0)

            # n1 preps: o1a (m0+m1) on q0, o1b (m2) on q1, m3 column pieces
            # on q2 (split into two concurrent transfer tracks) and q3.
            o3 = (MO + 3) * P
            preps = [
                (0, out[(MO + 0) * P:(MO + 2) * P], o1a[:], idx[:, 0:16], 256, N_TILE, None),
                (1, out[(MO + 2) * P:(MO + 3) * P], o1b[:], idx[:, 0:8], 128, N_TILE, None),
                (2, out[o3:o3 + P, 0:256], o1ca[:, :, 0:256], idx[:, 0:8], 128, 256, N_TILE),
                (2, out[o3:o3 + P, 256:ha], o1ca[:, :, 256:ha], idx[:, 0:8], 128, ha - 256, N_TILE),
                (3, out[o3:o3 + P, ha:], o1cb[:], idx[:, 0:8], 128, hb, N_TILE),
            ]
            for i, (q, dst, src, ix, n_idx, esz, estep) in enumerate(preps):
                nc.gpsimd.dma_scatter_add(
                    dst, src, ix, n_idx, n_idx, esz, elem_step=estep,
                    prepare_only=True, sem=sems[2 + i], queue_num=q)

            # --- n1 (pixels 512..1023): m-outer so m-groups retire
            # staggered; each trigger fires right after its staging drain.
            def mm_group(m, ps, js):
                for k in range(KO):
                    nc.tensor.matmul(ps[:], lhsT=lhsT(k, m), rhs=x1_sb[:, k, js],
                                     start=(k == 0), stop=(k == KO - 1))

            ps1 = [psum_pool.tile([P, N_TILE], _FP32, tag="ps", name=f"ps1_{m}")
                   for m in range(3)]
            mm_group(0, ps1[0], slice(0, N_TILE))
            nc.scalar.activation(o1a[:, 0], ps1[0][:],
                                 mybir.ActivationFunctionType.Identity,
                                 bias=b_sb[:, 0:1])
            mm_group(1, ps1[1], slice(0, N_TILE))
            nc.vector.tensor_scalar_add(o1a[:, 1], ps1[1][:], b_sb[:, 1:2])
            nc.gpsimd.trigger_dma(count=None, queue_num=0)
            mm_group(2, ps1[2], slice(0, N_TILE))
            nc.scalar.activation(o1b[:, 0], ps1[2][:],
                                 mybir.ActivationFunctionType.Identity,
                                 bias=b_sb[:, 2:3])
            nc.gpsimd.trigger_dma(count=None, queue_num=1)
            # m3: two column pieces; drains balanced ACT (320) / DVE (192).
            ps3a = psum_pool.tile([P, ha], _FP32, tag="ps", name="ps1_3a")
            mm_group(3, ps3a, slice(0, ha))
            nc.scalar.activation(o1ca[:, 0], ps3a[:],
                                 mybir.ActivationFunctionType.Identity,
                                 bias=b_sb[:, 3:4])
            nc.gpsimd.trigger_dma(count=None, queue_num=2)
            ps3b = psum_pool.tile([P, hb], _FP32, tag="ps", name="ps1_3b")
            mm_group(3, ps3b, slice(ha, N_TILE))
            nc.vector.tensor_scalar_add(o1cb[:, 0], ps3b[:], b_sb[:, 3:4])
            nc.gpsimd.trigger_dma(count=None, queue_num=3)


def _build_bass(mode="fp16s"):
    # Bacc (not plain Bass): its finalize() runs the legalization passes that
    # split multi-semaphore waits (TRN2 allows one sync wait per instruction).
    if mode == "fp16s":
        nc = bacc.Bacc(num_swdge_queues=4)
        _build_fp16s(nc)
    else:
        raise ValueError(mode)
    nc.finalize()
    return nc


def _pack_w(w2d):
    # [C, C] (transposed W_eff: w2d[c, o]) -> [P, KO*C] with [p, ko, o] layout
    return np.ascontiguousarray(
        w2d.reshape(KO, P, C).transpose(1, 0, 2)).reshape(P, KO * C)


def _pack_x(xm):
    # [B, C, HW] -> [B, N_TILES*P, KO*N_TILE] with [n, p, ko, j] layout
    t = xm.reshape(B, KO, P, N_TILES, N_TILE).transpose(0, 3, 2, 1, 4)
    return np.ascontiguousarray(t).reshape(B, N_TILES * P, KO * N_TILE)


_NC_CACHE = {}


def _get_nc(mode):
    if mode not in _NC_CACHE:
        _NC_CACHE[mode] = _build_bass(mode)
    return _NC_CACHE[mode]


MODE = "fp16s"

# replicated scatter-index table: sidx[p, s] = 16*s + (p % 16)
_SIDX = np.ascontiguousarray(np.tile(
    (np.arange(16)[:, None] + 16 * np.arange(16)[None, :]).astype(np.int16),
    (P // 16, 1)))


def kernel(x, qkv_w, qkv_b, out_w, out_b):
    x = np.asarray(x, dtype=np.float32)
    qkv_w = np.asarray(qkv_w, dtype=np.float32)
    qkv_b = np.asarray(qkv_b, dtype=np.float32)
    out_w = np.asarray(out_w, dtype=np.float32)
    out_b = np.asarray(out_b, dtype=np.float32)

    Bx, Cx, Hx, Wx = x.shape
    assert (Bx, Cx, Hx * Wx) == (B, C, HW), (x.shape,)

    # Host-side algebraic fusion (see module docstring).
    v_w = qkv_w[2 * C:3 * C]
    v_b = qkv_b[2 * C:3 * C]
    w_eff = out_w @ v_w                    # [C, C]
    b_eff = out_w @ v_b + out_b            # [C]

    bias_host = np.ascontiguousarray(b_eff.reshape(MO, P).T.astype(np.float32))
    wt = np.ascontiguousarray(w_eff.T)     # wt[c, o]
    w_dev = _pack_w(wt).astype(np.float16)
    xm = x.reshape(B, C, HW)
    x_pack = _pack_x(xm).astype(np.float16)

    nc = _get_nc(MODE)
    in_maps = []
    for b in range(B):
        x0 = x_pack[b, 0:P]
        wx = np.concatenate(
            [np.concatenate([w_dev[:, k * C:(k + 1) * C],
                             x0[:, k * N_TILE:(k + 1) * N_TILE]], axis=1)
             for k in range(KO)], axis=1)          # [P, KO*(C+N_TILE)]
        in_maps.append({
            "wx": np.ascontiguousarray(wx),
            "x1": np.ascontiguousarray(x_pack[b, P:2 * P]),
            "bias": bias_host,
            "sidx": _SIDX,
        })

    res = run_bass_kernel_spmd(nc, in_maps, core_ids=list(range(B)))

    # out rows [(n*MO + m)*P + p] hold out_core[m*P + p, n*N_TILE:(n+1)*N_TILE]
    out_dev = np.stack([res.results[i]["out"] for i in range(B)], axis=0)
    out_dev = out_dev.reshape(B, N_TILES, MO, P, N_TILE)
    out_full = out_dev.transpose(0, 2, 3, 1, 4).reshape(B, C, Hx, Wx)
    return np.ascontiguousarray(out_full.astype(np.float32))


# revision 4
# speedup vs baseline: 2.6901x; 1.0232x over previous
"""Trainium2 kernel for nn_MHAttention_15358803050646.

The reference module computes
    qkv = qkv_w @ x + qkv_b          (1x1 conv over channels)
    q, k, v = split(qkv)
    att = softmax(q @ k^T / sqrt(d_k))
    out = einsum('bnqk,bnqd->bnqd', att, v)      # <-- sums att over k
    out = out_w @ out + out_b

The einsum 'bnqk,bnqd->bnqd' multiplies v elementwise by the softmax
row-sum, which is identically 1.  The whole attention block is therefore
the identity on v, and the network collapses algebraically to

    out = out_w @ (v_w @ x + v_b) + out_b = W_eff @ x + b_eff

with v_w = qkv_w[1024:1536], v_b = qkv_b[1024:1536].  We fuse the two
channel matrices on the host (512x512x512 fp32, sub-millisecond) and run
a single 512x512 channel projection over all pixels on device.

Sharding: data-parallel over batch — B == 8 images, one per NeuronCore.
Per core: out[o, p] = sum_c W_eff[o, c] * x[c, p] + b_eff[o] with
C = 512 channels and HW = 1024 pixels, i.e. a 512x512x1024 matmul.

Kernel design (mode "fp16s", cost-model driven):
- Single fp16 matmul pass.  fp16 carries 11 mantissa bits; with fp32 PSUM
  accumulation the end-to-end relative error is ~4e-4, far inside the 2e-2
  gate, at 1 PE cycle/row (fp32 needs 4, a hi/lo fp16 split needs 3).
- fp16 outputs; the host upcasts to fp32.  Halves the store traffic.
- Inputs stream on the SP queue as four fused [w_k | x_k] 256KB chunks so
  one DMA enables a full contraction round, then the two x-halves of the
  second pixel tile.
- 5 warmup matmuls on a memset tile keep the PE busy from ~0.5us so the
  p-state ramp (0.65 -> 1.2 -> 2.4 GHz after 3us continuous busy) is over
  as early as possible; real matmuls start as soon as the first chunk
  lands (~0.77us; the first chunk is the k0 weights plus the first 128
  x columns, accumulated in separate PSUM tiles).
- Bias-add + PSUM->SBUF fp16 downcast alternates between the Activation
  and Vector engines so neither becomes the drain bottleneck.
- All output stores go through SWDGE prepare_only scatter-add descriptors
  (generated early on the idle Pool engine) fired by trigger_dma right
  after each staging drain.  This skips the HWDGE+DGE store pipeline
  (~1.2us) at the tail.  Scatter-add accumulates, so the out buffer is
  zero-filled first by two 512KB DMAs from a memset tile (off the
  critical path).  The last m-group is column-split 320/192 with drains
  balanced across ACT/DVE so the final trigger fires ~320ns after the
  last matmul.  Final CoreSim cost: 9897ns/core vs 26624ns baseline.
"""

import numpy as np

import concourse.mybir as mybir
import concourse.tile as tile
from concourse import bacc
from concourse.bass_utils import run_bass_kernel_spmd

P = 128          # SBUF partitions
C = 512          # model channels
HW = 1024        # pixels per image (32*32)
B = 8            # batch == number of cores
KO = C // P      # contraction chunks (4)
MO = C // P      # output-channel chunks (4)
N_TILE = 512     # pixels per PSUM tile (one fp32 PSUM bank)
N_TILES = HW // N_TILE

_FP32 = mybir.dt.float32
_FP16 = mybir.dt.float16

WARM_INSTS = 3   # warmup matmuls before real operands land
WARM_COLS = 114  # columns per warmup matmul (3 end right as chunk 1 lands)
M3A = 280        # column split of the last m-group (ACT 280 | DVE 232)
X0F = 128        # first-chunk split: [w_k0 | x00 cols 0:128] arrives first


def _build_fp16s(nc):
    """Single-pass fp16 kernel body (see module docstring)."""
    # Fused per-k chunks: [w_k (C cols) | x_n0_k (N_TILE cols)] so one DMA
    # enables a full k-round of the first pixel tile.
    wx = nc.declare_dram_parameter("wx", [P, KO * (C + N_TILE)], _FP16, isOutput=False)
    x1 = nc.declare_dram_parameter("x1", [P, KO * N_TILE], _FP16, isOutput=False)
    bias = nc.declare_dram_parameter("bias", [P, MO], _FP32, isOutput=False)
    # scatter index table, replicated over 16-partition groups:
    # sidx[p, s] = 16*s + (p % 16).  Host-provided: the device SWDGE ucode
    # reads the replicated layout (an iota over partitions 0:16 with -1
    # padding crashes it).
    sidx = nc.declare_dram_parameter("sidx", [P, 16], mybir.dt.int16, isOutput=False)
    # out[(n*MO + m)*P + p, j] = out_core[m*P + p, n*N_TILE + j]
    out = nc.declare_dram_parameter("out", [N_TILES * MO * P, N_TILE], _FP16, isOutput=True)

    wx_r = wx.rearrange("p (ko c) -> p ko c", ko=KO)
    x1_r = x1.rearrange("p (ko j) -> p ko j", ko=KO)
    ha, hb = M3A, N_TILE - M3A

    with tile.TileContext(nc) as tc:
        with (
            tc.tile_pool(name="wpool", bufs=1) as wpool,
            tc.tile_pool(name="opool", bufs=1) as opool,
            tc.tile_pool(name="psum", bufs=8, space="PSUM") as psum_pool,
        ):
            # --- warmup: PE busy from ~t=0.5us so the p-state ramp is done
            # by t=3.3us.
            wz = wpool.tile([P, max(P, WARM_COLS)], _FP16, tag="wz")
            nc.vector.memset(wz[:], 0.03125)
            ps_warm = psum_pool.tile([P, max(P, WARM_COLS)], _FP32, tag="ps",
                                     name="ps_warm")
            for _ in range(WARM_INSTS):
                nc.tensor.matmul(ps_warm[:, :WARM_COLS], lhsT=wz[:, :P],
                                 rhs=wz[:, :WARM_COLS], start=True, stop=True)

            # --- bias on the Pool queue (SWDGE), keeps HWDGE free.
            b_sb = wpool.tile([P, MO], _FP32, tag="bias")
            nc.gpsimd.dma_start(b_sb[:], bias[:])

            # --- input stream on the SP queue (HWDGE).
            wx_sb = [wpool.tile([P, C + N_TILE], _FP16, tag=f"wx{k}", name=f"wx{k}")
                     for k in range(KO)]
            # chunk 1 split: [w_k0 | x00 cols 0:X0F] lands ~270ns before the
            # full fused chunk would, so real matmuls start that much sooner
            # (the PE is half-speed until t=3.3us, so this moves the whole
            # schedule left).
            nc.sync.dma_start(wx_sb[0][:, :C + X0F], wx_r[:, 0, :C + X0F])
            nc.sync.dma_start(wx_sb[0][:, C + X0F:], wx_r[:, 0, C + X0F:])
            for k in range(1, KO):
                nc.sync.dma_start(wx_sb[k][:], wx_r[:, k])
            x1_sb = wpool.tile([P, KO, N_TILE], _FP16, tag="x1")
            nc.sync.dma_start(x1_sb[:, 0:2], x1_r[:, 0:2])
            nc.sync.dma_start(x1_sb[:, 2:4], x1_r[:, 2:4])

            def lhsT(k, m):
                return wx_sb[k][:, m * P:(m + 1) * P]

            # --- n0 (pixels 0..511): k-outer (stream-friendly), m-inner.
            # Columns 0:X0F accumulate in their own PSUM tiles (an fp32 PSUM
            # bank allows only one open accumulation group, so the early
            # column piece cannot share a bank with the rest).
            ps0a = [psum_pool.tile([P, X0F], _FP32, tag="ps", name=f"ps0a_{m}")
                    for m in range(MO)]
            ps0 = [psum_pool.tile([P, N_TILE - X0F], _FP32, tag="ps", name=f"ps0_{m}")
                   for m in range(MO)]
            for m in range(MO):
                nc.tensor.matmul(ps0a[m][:], lhsT=lhsT(0, m),
                                 rhs=wx_sb[0][:, C:C + X0F],
                                 start=True, stop=False)
            for m in range(MO):
                nc.tensor.matmul(ps0[m][:], lhsT=lhsT(0, m),
                                 rhs=wx_sb[0][:, C + X0F:],
                                 start=True, stop=False)
            for k in range(1, KO):
                for m in range(MO):
                    nc.tensor.matmul(ps0a[m][:], lhsT=lhsT(k, m),
                                     rhs=wx_sb[k][:, C:C + X0F],
                                     start=False, stop=(k == KO - 1))
                    nc.tensor.matmul(ps0[m][:], lhsT=lhsT(k, m),
                                     rhs=wx_sb[k][:, C + X0F:],
                                     start=False, stop=(k == KO - 1))

            # n0 drain: ACT m0/m2, DVE m1/m3, into one staging tile.
            o0 = opool.tile([P, MO, N_TILE], _FP16, tag="o0")
            nc.scalar.activation(o0[:, 0, 0:X0F], ps0a[0][:],
                                 mybir.ActivationFunctionType.Identity,
                                 bias=b_sb[:, 0:1])
            nc.scalar.activation(o0[:, 0, X0F:], ps0[0][:],
                                 mybir.ActivationFunctionType.Identity,
                                 bias=b_sb[:, 0:1])
            nc.vector.tensor_scalar_add(o0[:, 1, 0:X0F], ps0a[1][:], b_sb[:, 1:2])
            nc.vector.tensor_scalar_add(o0[:, 1, X0F:], ps0[1][:], b_sb[:, 1:2])
            nc.scalar.activation(o0[:, 2, 0:X0F], ps0a[2][:],
                                 mybir.ActivationFunctionType.Identity,
                                 bias=b_sb[:, 2:3])
            nc.scalar.activation(o0[:, 2, X0F:], ps0[2][:],
                                 mybir.ActivationFunctionType.Identity,
                                 bias=b_sb[:, 2:3])
            nc.vector.tensor_scalar_add(o0[:, 3, 0:X0F], ps0a[3][:], b_sb[:, 3:4])
            nc.vector.tensor_scalar_add(o0[:, 3, X0F:], ps0[3][:], b_sb[:, 3:4])

            # --- n1 staging tiles (scatter sources must be [128, g, elem]).
            o1a = opool.tile([P, 2, N_TILE], _FP16, tag="o1a")
            o1b = opool.tile([P, 1, N_TILE], _FP16, tag="o1b")
            o1ca = opool.tile([P, 1, ha], _FP16, tag="o1ca")
            o1cb = opool.tile([P, 1, hb], _FP16, tag="o1cb")

            # scatter indices (see sidx declaration).
            idx = opool.tile([P, 16], mybir.dt.int16, tag="sidx")
            nc.gpsimd.dma_start(idx[:], sidx[:])

            # scatter-add accumulates, so zero-fill the whole out buffer
            # first: two 512KB DMAs from a memset tile, early on the Pool
            # queue.  The preps' WAW attribution orders the scattered
            # writes after these.
            zt = opool.tile([P, 2, N_TILE], _FP16, tag="zt")
            nc.vector.memset(zt[:], 0)
            for zb in range(0, 2 * MO, 2):
                nc.gpsimd.dma_start(
                    out[zb * P:(zb + 2) * P].rearrange("(mo p) j -> p mo j", p=P),
                    zt[:])

            # n0 preps on queue 0, fired as soon as the n0 drains land.
            # (o1a reuses queue 0 afterwards: its prep is emitted after this
            # trigger, so the pending list is empty again.)
            sems = [nc.alloc_semaphore(f"sc{i}") for i in range(7)]
            nc.gpsimd.dma_scatter_add(
                out[0:2 * P], o0[:, 0:2], idx[:, 0:16], 256, 256, N_TILE,
                prepare_only=True, sem=sems[0], queue_num=0)
            nc.gpsimd.dma_scatter_add(
                out[2 * P:MO * P], o0[:, 2:4], idx[:, 0:16], 256, 256, N_TILE,
                prepare_only=True, sem=sems[1], queue_num=0)
            nc.gpsimd.trigger_dma(count=None, queue_num=0)

            # n1 preps: o1a (m0+m1) on q0, o1b (m2) on q1, m3 column pieces
            # on q2 (split into two concurrent transfer tracks) and q3.
            o3 = (MO + 3) * P
            preps = [
                (0, out[(MO + 0) * P:(MO + 2) * P], o1a[:], idx[:, 0:16], 256, N_TILE, None),
                (1, out[(MO + 2) * P:(MO + 3) * P], o1b[:], idx[:, 0:8], 128, N_TILE, None),
                (2, out[o3:o3 + P, 0:256], o1ca[:, :, 0:256], idx[:, 0:8], 128, 256, N_TILE),
                (2, out[o3:o3 + P, 256:ha], o1ca[:, :, 256:ha], idx[:, 0:8], 128, ha - 256, N_TILE),
                (3, out[o3:o3 + P, ha:], o1cb[:], idx[:, 0:8], 128, hb, N_TILE),
            ]
            for i, (q, dst, src, ix, n_idx, esz, estep) in enumerate(preps):
                nc.gpsimd.dma_scatter_add(
                    dst, src, ix, n_idx, n_idx, esz, elem_step=estep,
                    prepare_only=True, sem=sems[2 + i], queue_num=q)

            # --- n1 (pixels 512..1023): m-outer so m-groups retire
            # staggered; each trigger fires right after its staging drain.
            def mm_group(m, ps, js):
                for k in range(KO):
                    nc.tensor.matmul(ps[:], lhsT=lhsT(k, m), rhs=x1_sb[:, k, js],
                                     start=(k == 0), stop=(k == KO - 1))

            ps1 = [psum_pool.tile([P, N_TILE], _FP32, tag="ps", name=f"ps1_{m}")
                   for m in range(3)]
            mm_group(0, ps1[0], slice(0, N_TILE))
            nc.scalar.activation(o1a[:, 0], ps1[0][:],
                                 mybir.ActivationFunctionType.Identity,
                                 bias=b_sb[:, 0:1])
            mm_group(1, ps1[1], slice(0, N_TILE))
            nc.vector.tensor_scalar_add(o1a[:, 1], ps1[1][:], b_sb[:, 1:2])
            nc.gpsimd.trigger_dma(count=None, queue_num=0)
            mm_group(2, ps1[2], slice(0, N_TILE))
            nc.scalar.activation(o1b[:, 0], ps1[2][:],
                                 mybir.ActivationFunctionType.Identity,
                                 bias=b_sb[:, 2:3])
            nc.gpsimd.trigger_dma(count=None, queue_num=1)
            # m3: two column pieces; drains balanced ACT (320) / DVE (192).
            ps3a = psum_pool.tile([P, ha], _FP32, tag="ps", name="ps1_3a")
            mm_group(3, ps3a, slice(0, ha))
            nc.scalar.activation(o1ca[:, 0], ps3a[:],
                                 mybir.ActivationFunctionType.Identity,
                                 bias=b_sb[:, 3:4])
            nc.gpsimd.trigger_dma(count=None, queue_num=2)
            ps3b = psum_pool.tile([P, hb], _FP32, tag="ps", name="ps1_3b")
            mm_group(3, ps3b, slice(ha, N_TILE))
            nc.vector.tensor_scalar_add(o1cb[:, 0], ps3b[:], b_sb[:, 3:4])
            nc.gpsimd.trigger_dma(count=None, queue_num=3)


def _build_bass(mode="fp16s"):
    # Bacc (not plain Bass): its finalize() runs the legalization passes that
    # split multi-semaphore waits (TRN2 allows one sync wait per instruction).
    if mode == "fp16s":
        nc = bacc.Bacc(num_swdge_queues=4)
        _build_fp16s(nc)
    else:
        raise ValueError(mode)
    nc.finalize()
    return nc


def _pack_w(w2d):
    # [C, C] (transposed W_eff: w2d[c, o]) -> [P, KO*C] with [p, ko, o] layout
    return np.ascontiguousarray(
        w2d.reshape(KO, P, C).transpose(1, 0, 2)).reshape(P, KO * C)


def _pack_x(xm):
    # [B, C, HW] -> [B, N_TILES*P, KO*N_TILE] with [n, p, ko, j] layout
    t = xm.reshape(B, KO, P, N_TILES, N_TILE).transpose(0, 3, 2, 1, 4)
    return np.ascontiguousarray(t).reshape(B, N_TILES * P, KO * N_TILE)


_NC_CACHE = {}


def _get_nc(mode):
    if mode not in _NC_CACHE:
        _NC_CACHE[mode] = _build_bass(mode)
    return _NC_CACHE[mode]


MODE = "fp16s"

# replicated scatter-index table: sidx[p, s] = 16*s + (p % 16)
_SIDX = np.ascontiguousarray(np.tile(
    (np.arange(16)[:, None] + 16 * np.arange(16)[None, :]).astype(np.int16),
    (P // 16, 1)))


def kernel(x, qkv_w, qkv_b, out_w, out_b):
    x = np.asarray(x, dtype=np.float32)
    qkv_w = np.asarray(qkv_w, dtype=np.float32)
    qkv_b = np.asarray(qkv_b, dtype=np.float32)
    out_w = np.asarray(out_w, dtype=np.float32)
    out_b = np.asarray(out_b, dtype=np.float32)

    Bx, Cx, Hx, Wx = x.shape
    assert (Bx, Cx, Hx * Wx) == (B, C, HW), (x.shape,)

    # Host-side algebraic fusion (see module docstring).
    v_w = qkv_w[2 * C:3 * C]
    v_b = qkv_b[2 * C:3 * C]
    w_eff = out_w @ v_w                    # [C, C]
    b_eff = out_w @ v_b + out_b            # [C]

    bias_host = np.ascontiguousarray(b_eff.reshape(MO, P).T.astype(np.float32))
    wt = np.ascontiguousarray(w_eff.T)     # wt[c, o]
    w_dev = _pack_w(wt).astype(np.float16)
    xm = x.reshape(B, C, HW)
    x_pack = _pack_x(xm).astype(np.float16)

    nc = _get_nc(MODE)
    in_maps = []
    for b in range(B):
        x0 = x_pack[b, 0:P]
        wx = np.concatenate(
            [np.concatenate([w_dev[:, k * C:(k + 1) * C],
                             x0[:, k * N_TILE:(k + 1) * N_TILE]], axis=1)
             for k in range(KO)], axis=1)          # [P, KO*(C+N_TILE)]
        in_maps.append({
            "wx": np.ascontiguousarray(wx),
            "x1": np.ascontiguousarray(x_pack[b, P:2 * P]),
            "bias": bias_host,
            "sidx": _SIDX,
        })

    res = run_bass_kernel_spmd(nc, in_maps, core_ids=list(range(B)))

    # out rows [(n*MO + m)*P + p] hold out_core[m*P + p, n*N_TILE:(n+1)*N_TILE]
    out_dev = np.stack([res.results[i]["out"] for i in range(B)], axis=0)
    out_dev = out_dev.reshape(B, N_TILES, MO, P, N_TILE)
    out_full = out_dev.transpose(0, 2, 3, 1, 4).reshape(B, C, Hx, Wx)
    return np.ascontiguousarray(out_full.astype(np.float32))
